# revision 13
# baseline (speedup 1.0000x reference)
"""Trainium2 Bass kernel for CustomCombinedLoss (weighted BCE sum + MultiMarginLoss).

loss = -sum(w * (pos_t*log(p) + (1-pos_t)*log(1-p)))          # w=2 for target==0
     + sum_{i: target_i>0} (1/C) * sum_{j != y_i} max(0, margin - x[i,y_i] + x[i,j])

Sharding: pure data parallel over the batch dim, B=16384 rows -> 8 cores x 2048 rows.
Each core computes a partial scalar loss; host sums the 8 partials.

v2 design (from the 43-49us fp16 baseline):
  - predictions stream as fp8 e3m4 (range +-15.5, 4-bit mantissa): halves HBM
    traffic to ~4.2 MB/core, a ~12.5 us DMA floor at ~340 GB/s.  Margin-term
    error from fp8 quantization is ~1e-4 relative (vs the 2e-2 gate).
  - all O(B) per-row parameters (pos, 2*pos-2, margin-xy biases, the PE-path
    max() thresholds and their corrections) are host-precomputed and arrive in
    one small f32 side DMA.  This removes the gpsimd ap_gather xy-extraction
    entirely - whose one-time ~6us Q7 IRAM library load was the baseline's
    critical-path bottleneck (first hinge op could not start until ~16us).
  - hinge work split across three engine paths, balanced to measured fp8
    per-tile costs ([128,2048] tile):
      ACT  (~2.19us): activation(Relu, bias=margin-xy, accum_out) -> row sums
      PE   (~1.13us DVE + ~1.0-1.8us PE): DVE tensor_scalar max -> fp8 junk,
           PE ones-stationary matmuls column-sum the junk into one [1,512]
           PSUM bank accumulated across all PE tiles; per-row corrections are
           host-precomputed constants.  The max() threshold is pre-rounded to
           fp8 on the host so the junk output cast is exact.
      CR   (tail only): DVE tensor_scalar(max, add) cache-reduce, where
           accum = scalar2_seed + sum_j max(x_j, scalar1)  (seed semantics).
    The last tile is column-split ACT|CR so both engines finish ~1.2us after
    the final (quarter-size) DMA chunk lands.
  - per-row loss assembled on DVE over a [128, 17] grid (16 tiles + the
    split-CR slot), reduced cross-partition via a tiny f32 matmul; the PE
    colsum bank is reduced by one ACT copy-accum.

Layout: row r = g*128 + p of the shard lives at partition p, grid column g.
pred DRAM is [128, 16*2048] fp8: tile g at columns [g*2048, (g+1)*2048).
"""

from contextlib import ExitStack

import ml_dtypes
import numpy as np

import concourse.bacc as bacc
import concourse.bass as bass
import concourse.mybir as mybir
import concourse.tile as tile
from concourse.bass_utils import run_bass_kernel_spmd

WEIGHT = 2.0
MARGIN = 0.5
B, C = 16384, 2048
NCORES = 8
BS = B // NCORES          # rows per core
P = 128                   # partitions
T = BS // P               # row tiles per core (16)
TX = T + 1                # grid cols incl. the split-CR accumulator slot
F32 = mybir.dt.float32
F16 = mybir.dt.float16
F8 = mybir.dt.float8e3
NPF8 = ml_dtypes.float8_e3m4

AluOp = mybir.AluOpType
ActFn = mybir.ActivationFunctionType
AxisList = mybir.AxisListType

# tile -> engine path assignment (tunable)
ACT_TILES = frozenset({0, 3, 6, 9, 12})
PE_TILES = frozenset({1, 2, 4, 5, 7, 8, 10, 11, 13, 14})
CR_TILES = frozenset()
SPLIT_TILE = 15
SPLIT_COL = 1280          # [0:SPLIT_COL] -> ACT, [SPLIT_COL:C] -> CR
NCR = C - SPLIT_COL

# DMA chunk schedule: (first_tile, n_tiles).  Uniform 512KB chunks: each
# chunk's completion semaphore waits on the slowest of the 16 SDMA engines
# (engine 15 chronically lags ~1.5-2.5us), so many small chunks multiply
# sem-wait bubbles while big chunks delay the pipeline head.
CHUNKS = ((0, 1), (1, 1), (2, 2), (4, 2), (6, 2), (8, 2), (10, 2), (12, 2),
          (14, 2))

# paux column offsets (all f32)
O_PPROB = 0               # [T]
O_POS = O_PPROB + T       # [TX]
O_C2 = O_POS + TX         # [T]
O_BIASA = O_C2 + T        # [T]
O_NB2 = O_BIASA + T       # [T]
O_ACC = O_NB2 + T         # [TX]
O_H3 = O_ACC + TX         # [TX]
O_S1CR = O_H3 + TX        # [1]
O_S2CR = O_S1CR + 1       # [1]
NAUX = O_S2CR + 1


def _loss_program(nc: bass.Bass, tc: "tile.TileContext", pred, paux, out):
    ctx = ExitStack()
    with ctx:
        small_pool = ctx.enter_context(tc.tile_pool(name="small", bufs=1))
        pred_pool = ctx.enter_context(tc.tile_pool(name="pred", bufs=len(CHUNKS)))
        psum_pool = ctx.enter_context(tc.tile_pool(name="psum", bufs=1, space="PSUM"))

        # paux rides first on the sync HWDGE ring: it gates every compute op,
        # and on the scalar/ACT ring its completion wait also trapped the
        # second ACT_TABLE_LOAD behind it (first Relu slipped ~2us).
        aux_t = small_pool.tile([P, NAUX], F32)
        nc.sync.dma_start(aux_t[:], paux[:])
        pprob = aux_t[:, O_PPROB : O_PPROB + T]
        pos_x = aux_t[:, O_POS : O_POS + TX]
        c2 = aux_t[:, O_C2 : O_C2 + T]
        acc = aux_t[:, O_ACC : O_ACC + TX]
        h3_x = aux_t[:, O_H3 : O_H3 + TX]

        ones8 = small_pool.tile([P, 1], F8)
        nc.vector.memset(ones8[:], 1.0)
        ones_t = small_pool.tile([P, 1], F32)
        nc.vector.memset(ones_t[:], 1.0)
        lp_x = small_pool.tile([P, TX], F32)
        nc.vector.memset(lp_x[:], 0.0)
        dh_x = small_pool.tile([P, TX], F32)
        nc.vector.memset(dh_x[:], 0.0)

        # ---- BCE row terms (off the critical path; Ln first so the ACT
        # table set containing Ln loads once, early).  The dummy Ln on a
        # memset tile has no DMA dependency: walrus places the ~1.3us
        # ACT_TABLE_LOAD right before the first Ln, and without the dummy
        # that position sits behind the paux-DMA semaphore wait.
        warm = small_pool.tile([P, 1], F32)
        nc.vector.memset(warm[:], 1.0)
        warm2 = small_pool.tile([P, 1], F32)
        nc.scalar.activation(warm2[:], warm[:], ActFn.Ln)
        nc.scalar.activation(lp_x[:, 0:T], pprob, ActFn.Ln)
        q_t = small_pool.tile([P, T], F32)
        nc.vector.tensor_scalar(q_t[:], pprob, -1.0, 1.0, AluOp.mult, AluOp.add)
        lq_t = small_pool.tile([P, T], F32)
        nc.scalar.activation(lq_t[:], q_t[:], ActFn.Ln)
        d_t = small_pool.tile([P, T], F32)
        nc.vector.tensor_mul(d_t[:], c2, lq_t[:])
        # dh = c2*lq + h3 - pos*lp, all folded early so the post-stream chain
        # is just t1 = (acc/C)*pos; e2 = t1 + dh; rowsum.  (col T stays h3's 0)
        nc.vector.tensor_add(dh_x[:, 0:T], d_t[:], h3_x[:, 0:T])
        nc.vector.tensor_copy(dh_x[:, T:TX], h3_x[:, T:TX])
        plq_x = small_pool.tile([P, TX], F32)
        nc.vector.tensor_mul(plq_x[:], pos_x, lp_x[:])
        nc.vector.tensor_tensor(dh_x[:], dh_x[:], plq_x[:], AluOp.subtract)

        # scratch outputs (never read)
        jact = small_pool.tile([P, C], F8)
        jcr = small_pool.tile([P, NCR], F16)
        jpes = [
            small_pool.tile([P, C], F8, name=f"jpe{i}") for i in range(3)
        ]

        colsum_ps = psum_pool.tile([1, 512], F32)
        first_pe = min(PE_TILES)
        last_pe = max(PE_TILES)

        # ---- stream chunks
        npe = 0
        for g0, ntiles in CHUNKS:
            st = pred_pool.tile([P, 2 * C], F8, tag="pred")
            nc.sync.dma_start(
                st[:, 0 : ntiles * C], pred[:, g0 * C : (g0 + ntiles) * C]
            )
            for b in range(ntiles):
                g = g0 + b
                blk = st[:, b * C : (b + 1) * C]
                gg = slice(g, g + 1)
                if g in ACT_TILES:
                    nc.scalar.activation(
                        jact[:], blk, ActFn.Relu,
                        bias=aux_t[:, O_BIASA + g : O_BIASA + g + 1],
                        scale=1.0, accum_out=acc[:, gg],
                    )
                elif g in PE_TILES:
                    jpe = jpes[npe % 3]
                    npe += 1
                    nc.vector.tensor_scalar(
                        jpe[:], blk, aux_t[:, O_NB2 + g : O_NB2 + g + 1],
                        None, AluOp.max,
                    )
                    for k in range(4):
                        csl = slice(k * 512, (k + 1) * 512)
                        nc.tensor.matmul(
                            colsum_ps[:], ones8[:], jpe[:, csl],
                            start=(g == first_pe and k == 0),
                            stop=(g == last_pe and k == 3),
                        )
                elif g in CR_TILES:
                    nc.vector.tensor_scalar(
                        jcr[:, 0:NCR], blk[:, 0:NCR],
                        aux_t[:, O_S1CR : O_S1CR + 1],
                        aux_t[:, O_S2CR : O_S2CR + 1],
                        AluOp.max, AluOp.add, accum_out=acc[:, gg],
                    )
                else:  # SPLIT_TILE
                    nc.scalar.activation(
                        jact[:, 0:SPLIT_COL], blk[:, 0:SPLIT_COL], ActFn.Relu,
                        bias=aux_t[:, O_BIASA + g : O_BIASA + g + 1],
                        scale=1.0, accum_out=acc[:, gg],
                    )
                    nc.vector.tensor_scalar(
                        jcr[:], blk[:, SPLIT_COL:C],
                        aux_t[:, O_S1CR : O_S1CR + 1],
                        aux_t[:, O_S2CR : O_S2CR + 1],
                        AluOp.max, AluOp.add, accum_out=acc[:, T : T + 1],
                    )

        # ---- epilogue
        # row_total = (acc/C)*pos + dh; dh already folds -pos*lp, c2*lq, the
        # -pos*M/C self-term and the PE-path neg-row correction.
        t1_x = small_pool.tile([P, TX], F32)
        nc.vector.scalar_tensor_tensor(
            t1_x[:], acc, 1.0 / C, pos_x, AluOp.mult, AluOp.mult
        )
        e2_x = small_pool.tile([P, TX], F32)
        nc.vector.tensor_add(e2_x[:], t1_x[:], dh_x[:])
        rowred = small_pool.tile([P, 1], F32)
        nc.vector.reduce_sum(rowred[:], e2_x[:], axis=AxisList.X)
        # PE colsum bank -> scalar on ACT (copy with accumulate), then inject
        # cs/C into rowred's partition 0 so the cross-partition matmul yields
        # the finished partial directly.
        cs_junk = small_pool.tile([1, 512], F32)
        cs_sc = small_pool.tile([1, 1], F32)
        nc.scalar.activation(
            cs_junk[:], colsum_ps[:], ActFn.Copy, accum_out=cs_sc[:]
        )
        nc.vector.scalar_tensor_tensor(
            rowred[0:1, 0:1], cs_sc[:], 1.0 / C, rowred[0:1, 0:1],
            AluOp.mult, AluOp.add,
        )
        # cross-partition sum via PE: rowred[128,1].T @ ones[128,1] -> [1,1]
        total_ps = psum_pool.tile([1, 1], F32)
        nc.tensor.matmul(total_ps[:], rowred[:], ones_t[:], start=True, stop=True)
        total = small_pool.tile([1, 1], F32)
        nc.vector.tensor_copy(total[:], total_ps[:])
        nc.sync.dma_start(out[:], total[:])


def build_nc() -> bass.Bass:
    nc = bacc.Bacc("TRN2", target_bir_lowering=False, debug=False, num_devices=NCORES)
    pred = nc.dram_tensor("pred", [P, T * C], F8, kind="ExternalInput").ap()
    paux = nc.dram_tensor("paux", [P, NAUX], F32, kind="ExternalInput").ap()
    out = nc.dram_tensor("out", [1, 1], F32, kind="ExternalOutput").ap()
    with tile.TileContext(nc) as tc:
        _loss_program(nc, tc, pred, paux, out)
    nc.compile()
    return nc


def make_in_maps(positive_prob, predictions, target):
    """Shard full inputs into per-core input maps (host-side prep only)."""
    pp_all = np.asarray(positive_prob, dtype=np.float32)
    tg_all = np.asarray(target).astype(np.int64)
    pr_all = np.asarray(predictions, dtype=np.float32)
    M = np.float32(MARGIN)
    in_maps = []
    for i in range(NCORES):
        sl = slice(i * BS, (i + 1) * BS)
        pr = pr_all[sl]                                   # [BS, C] f32
        pr8 = pr.astype(NPF8)
        # [BS, C] -> [P, T*C]: row g*P + p -> partition p, cols [g*C,(g+1)*C)
        pred8 = np.ascontiguousarray(
            pr8.reshape(T, P, C).transpose(1, 0, 2).reshape(P, T * C)
        )
        pp = pp_all[sl].reshape(T, P).T                   # [P, T]
        tg = tg_all[sl]
        pos = (tg != 0).astype(np.float32).reshape(T, P).T
        c2 = 2.0 * pos - 2.0
        y = np.maximum(tg - 1, 0)
        xy = pr[np.arange(BS), y].reshape(T, P).T         # exact f32 [P, T]
        biasA = (M - xy).astype(np.float32)
        # PE path: threshold nb2 pre-rounded to fp8 (so the fp8 junk cast is
        # exact); 8.0 for negative rows (all junk elems become exactly 8.0,
        # the largest power of two under the e3m4 max of 15.5).
        nb2q = (xy - M).astype(NPF8).astype(np.float32)
        nb2_all = np.where(pos > 0, nb2q, np.float32(8.0))
        pe_mask = np.zeros((1, T), dtype=np.float32)
        for g in PE_TILES:
            pe_mask[0, g] = 1.0
        nb2 = nb2_all * pe_mask
        acc0 = np.zeros((P, TX), dtype=np.float32)
        acc0[:, 0:T] = -np.float32(C) * nb2
        # h3 = 0.5*c2*nb2 - pos*M/C  (PE neg-row cancel + margin self-term)
        h3 = np.zeros((P, TX), dtype=np.float32)
        h3[:, 0:T] = 0.5 * c2 * nb2 - pos * (M / np.float32(C))
        pos_ext = np.concatenate([pos, pos[:, SPLIT_TILE : SPLIT_TILE + 1]], axis=1)
        s1cr = (xy - M)[:, SPLIT_TILE : SPLIT_TILE + 1].astype(np.float32)
        s2cr = np.float32(NCR) * (M - xy)[:, SPLIT_TILE : SPLIT_TILE + 1]
        paux = np.ascontiguousarray(
            np.concatenate(
                [pp, pos_ext, c2, biasA, nb2, acc0, h3, s1cr, s2cr], axis=1
            ).astype(np.float32)
        )
        in_maps.append({"pred": pred8, "paux": paux})
    return in_maps


_NC_CACHE = []


def kernel(positive_prob, predictions, target):
    in_maps = make_in_maps(positive_prob, predictions, target)
    if not _NC_CACHE:
        _NC_CACHE.append(build_nc())
    nc = _NC_CACHE[0]
    res = run_bass_kernel_spmd(nc, in_maps, list(range(NCORES)))
    total = np.float32(0.0)
    for r in res.results:
        total += np.float32(r["out"][0, 0])
    return np.asarray(total, dtype=np.float32)


# revision 21
# speedup vs baseline: 1.0620x; 1.0620x over previous
"""Trainium2 Bass kernel for CustomCombinedLoss (weighted BCE sum + MultiMarginLoss).

loss = -sum(w * (pos_t*log(p) + (1-pos_t)*log(1-p)))          # w=2 for target==0
     + sum_{i: target_i>0} (1/C) * sum_{j != y_i} max(0, margin - x[i,y_i] + x[i,j])

Sharding: pure data parallel over the batch dim, B=16384 rows -> 8 cores x 2048 rows.
Each core computes a partial scalar loss; host sums the 8 partials.

Design (measured ~30-31us vs the 43-49us fp16 gather-based baseline):
  - predictions stream as fp8 e3m4 (range +-15.5, 4-bit mantissa): halves HBM
    traffic to ~4.2 MB/core, a ~12.5 us DMA floor at ~340 GB/s.  Margin-term
    error from fp8 quantization is ~1e-4 relative (vs the 2e-2 gate).
  - all O(B) per-row parameters (pos, 2*pos-2, margin-xy biases, the PE-path
    max() thresholds and their corrections) are host-precomputed and arrive in
    one small f32 side DMA.  This removes the gpsimd ap_gather xy-extraction
    entirely - whose one-time ~6us Q7 IRAM library load was the baseline's
    critical-path bottleneck (first hinge op could not start until ~16us).
  - hinge work split across three engine paths, balanced to measured fp8
    per-tile costs ([128,2048] tile):
      ACT  (~2.19us): activation(Relu, bias=margin-xy, accum_out) -> row sums
      PE   (~1.2us DVE + ~1.0-1.8us PE): DVE tensor_scalar max -> fp8 junk,
           PE ones-stationary matmuls column-sum the junk into one [1,512]
           PSUM bank accumulated across all PE tiles; per-row corrections are
           host-precomputed constants.  The max() threshold is pre-rounded to
           fp8 on the host so the junk output cast is exact.  The PE ramps to
           its full p-state (216ns/matmul) because the matmul queue stays
           continuously busy.
      CR   (tail of the last tile only): DVE tensor_scalar(max, add)
           cache-reduce, where accum = scalar2_seed + sum_j max(x_j, scalar1)
           (seed semantics, verified in CoreSim).
    The last tile is column-split ACT|CR so both engines finish ~1us after
    the final DMA chunk lands.
  - per-row loss assembled on DVE over a [128, 17] grid (16 tiles + the
    split-CR slot): row_total = (acc/C)*pos + dh, with dh = c2*lq + h3 -
    pos*lp folded early so the post-stream chain is 3 ops + a bf16
    cross-partition matmul.  The PE colsum bank is reduced by one ACT
    copy-accum into out[0,1]; the host adds out[0,0] + out[0,1]/C per core
    so the two end chains only meet at the final [1,2] DMA.
  - DMA: 8 chunks of 2 tiles (512KB).  Every chunk's completion semaphore
    waits on the slowest of the 16 SDMA engines (engine 15 chronically lags
    1.5-2.5us and crawls on its first descriptors), so finer chunking
    multiplies sem-wait bubbles; the first chunk rides the scalar HWDGE ring
    so its descriptor generation runs parallel to paux's on the sync ring.
    A dummy Ln on a memset tile hoists the ~1.3us ACT_TABLE_LOAD off the
    paux critical path.

Layout: row r = g*128 + p of the shard lives at partition p, grid column g.
pred DRAM is [128, 16*2048] fp8: tile g at columns [g*2048, (g+1)*2048).
"""

from contextlib import ExitStack

import ml_dtypes
import numpy as np

import concourse.bacc as bacc
import concourse.bass as bass
import concourse.mybir as mybir
import concourse.tile as tile
from concourse.bass_utils import run_bass_kernel_spmd

WEIGHT = 2.0
MARGIN = 0.5
B, C = 16384, 2048
NCORES = 8
BS = B // NCORES          # rows per core
P = 128                   # partitions
T = BS // P               # row tiles per core (16)
TX = T + 1                # grid cols incl. the split-CR accumulator slot
F32 = mybir.dt.float32
F16 = mybir.dt.float16
F8 = mybir.dt.float8e3
NPF8 = ml_dtypes.float8_e3m4

AluOp = mybir.AluOpType
ActFn = mybir.ActivationFunctionType
AxisList = mybir.AxisListType

# tile -> engine path assignment (tunable)
ACT_TILES = frozenset({0, 3, 6, 9, 12})
PE_TILES = frozenset({1, 2, 4, 5, 7, 8, 10, 11, 13, 14})
CR_TILES = frozenset()
SPLIT_TILE = 15
SPLIT_COL = 1408          # [0:SPLIT_COL] -> ACT, [SPLIT_COL:C] -> CR
NCR = C - SPLIT_COL

# DMA chunk schedule: (first_tile, n_tiles).  Uniform 512KB chunks: each
# chunk's completion semaphore waits on the slowest of the 16 SDMA engines
# (engine 15 chronically lags ~1.5-2.5us), so many small chunks multiply
# sem-wait bubbles while big chunks delay the pipeline head.
CHUNKS = ((0, 2), (2, 2), (4, 2), (6, 2), (8, 2), (10, 2), (12, 2),
          (14, 2))

# paux column offsets (all f32)
O_PPROB = 0               # [T]
O_POS = O_PPROB + T       # [TX]
O_C2 = O_POS + TX         # [T]
O_BIASA = O_C2 + T        # [T]
O_NB2 = O_BIASA + T       # [T]
O_ACC = O_NB2 + T         # [TX]
O_H3 = O_ACC + TX         # [TX]
O_S1CR = O_H3 + TX        # [1]
O_S2CR = O_S1CR + 1       # [1]
NAUX = O_S2CR + 1


def _loss_program(nc: bass.Bass, tc: "tile.TileContext", pred, paux, out):
    ctx = ExitStack()
    with ctx:
        small_pool = ctx.enter_context(tc.tile_pool(name="small", bufs=1))
        pred_pool = ctx.enter_context(tc.tile_pool(name="pred", bufs=len(CHUNKS)))
        psum_pool = ctx.enter_context(tc.tile_pool(name="psum", bufs=1, space="PSUM"))

        # paux rides first on the sync HWDGE ring: it gates every compute op,
        # and on the scalar/ACT ring its completion wait also trapped the
        # second ACT_TABLE_LOAD behind it (first Relu slipped ~2us).
        aux_t = small_pool.tile([P, NAUX], F32)
        nc.sync.dma_start(aux_t[:], paux[:])
        pprob = aux_t[:, O_PPROB : O_PPROB + T]
        pos_x = aux_t[:, O_POS : O_POS + TX]
        c2 = aux_t[:, O_C2 : O_C2 + T]
        acc = aux_t[:, O_ACC : O_ACC + TX]
        h3_x = aux_t[:, O_H3 : O_H3 + TX]

        ones8 = small_pool.tile([P, 1], F8)
        nc.vector.memset(ones8[:], 1.0)
        lp_x = small_pool.tile([P, TX], F32)
        nc.vector.memset(lp_x[:], 0.0)
        dh_x = small_pool.tile([P, TX], F32)
        nc.vector.memset(dh_x[:], 0.0)

        # ---- BCE row terms (off the critical path; Ln first so the ACT
        # table set containing Ln loads once, early).  The dummy Ln on a
        # memset tile has no DMA dependency: walrus places the ~1.3us
        # ACT_TABLE_LOAD right before the first Ln, and without the dummy
        # that position sits behind the paux-DMA semaphore wait.
        warm = small_pool.tile([P, 1], F32)
        nc.vector.memset(warm[:], 1.0)
        warm2 = small_pool.tile([P, 1], F32)
        nc.scalar.activation(warm2[:], warm[:], ActFn.Ln)
        nc.scalar.activation(lp_x[:, 0:T], pprob, ActFn.Ln)
        q_t = small_pool.tile([P, T], F32)
        nc.vector.tensor_scalar(q_t[:], pprob, -1.0, 1.0, AluOp.mult, AluOp.add)
        lq_t = small_pool.tile([P, T], F32)
        nc.scalar.activation(lq_t[:], q_t[:], ActFn.Ln)
        d_t = small_pool.tile([P, T], F32)
        nc.vector.tensor_mul(d_t[:], c2, lq_t[:])
        # dh = c2*lq + h3 - pos*lp, all folded early so the post-stream chain
        # is just t1 = (acc/C)*pos; e2 = t1 + dh; rowsum.  (col T stays h3's 0)
        nc.vector.tensor_add(dh_x[:, 0:T], d_t[:], h3_x[:, 0:T])
        nc.vector.tensor_copy(dh_x[:, T:TX], h3_x[:, T:TX])
        plq_x = small_pool.tile([P, TX], F32)
        nc.vector.tensor_mul(plq_x[:], pos_x, lp_x[:])
        nc.vector.tensor_tensor(dh_x[:], dh_x[:], plq_x[:], AluOp.subtract)

        # scratch outputs (never read)
        jact = small_pool.tile([P, C], F8)
        jcr = small_pool.tile([P, NCR], F16)
        jpes = [
            small_pool.tile([P, C], F8, name=f"jpe{i}") for i in range(3)
        ]

        colsum_ps = psum_pool.tile([1, 512], F32)
        first_pe = min(PE_TILES)
        last_pe = max(PE_TILES)

        # ---- stream chunks
        npe = 0
        for ci, (g0, ntiles) in enumerate(CHUNKS):
            st = pred_pool.tile([P, 2 * C], F8, tag="pred")
            eng = nc.scalar if ci == 0 else nc.sync
            eng.dma_start(
                st[:, 0 : ntiles * C], pred[:, g0 * C : (g0 + ntiles) * C]
            )
            for b in range(ntiles):
                g = g0 + b
                blk = st[:, b * C : (b + 1) * C]
                gg = slice(g, g + 1)
                if g in ACT_TILES:
                    nc.scalar.activation(
                        jact[:], blk, ActFn.Relu,
                        bias=aux_t[:, O_BIASA + g : O_BIASA + g + 1],
                        scale=1.0, accum_out=acc[:, gg],
                    )
                elif g in PE_TILES:
                    jpe = jpes[npe % 3]
                    npe += 1
                    nc.vector.tensor_scalar(
                        jpe[:], blk, aux_t[:, O_NB2 + g : O_NB2 + g + 1],
                        None, AluOp.max,
                    )
                    for k in range(4):
                        csl = slice(k * 512, (k + 1) * 512)
                        nc.tensor.matmul(
                            colsum_ps[:], ones8[:], jpe[:, csl],
                            start=(g == first_pe and k == 0),
                            stop=(g == last_pe and k == 3),
                        )
                elif g in CR_TILES:
                    nc.vector.tensor_scalar(
                        jcr[:, 0:NCR], blk[:, 0:NCR],
                        aux_t[:, O_S1CR : O_S1CR + 1],
                        aux_t[:, O_S2CR : O_S2CR + 1],
                        AluOp.max, AluOp.add, accum_out=acc[:, gg],
                    )
                else:  # SPLIT_TILE
                    nc.scalar.activation(
                        jact[:, 0:SPLIT_COL], blk[:, 0:SPLIT_COL], ActFn.Relu,
                        bias=aux_t[:, O_BIASA + g : O_BIASA + g + 1],
                        scale=1.0, accum_out=acc[:, gg],
                    )
                    nc.vector.tensor_scalar(
                        jcr[:], blk[:, SPLIT_COL:C],
                        aux_t[:, O_S1CR : O_S1CR + 1],
                        aux_t[:, O_S2CR : O_S2CR + 1],
                        AluOp.max, AluOp.add, accum_out=acc[:, T : T + 1],
                    )

        # ---- epilogue
        # row_total = (acc/C)*pos + dh; dh already folds -pos*lp, c2*lq, the
        # -pos*M/C self-term and the PE-path neg-row correction.
        t1_x = small_pool.tile([P, TX], F32)
        nc.vector.scalar_tensor_tensor(
            t1_x[:], acc, 1.0 / C, pos_x, AluOp.mult, AluOp.mult
        )
        e2_x = small_pool.tile([P, TX], F32)
        nc.vector.tensor_add(e2_x[:], t1_x[:], dh_x[:])
        # bf16 rowred: the PE f32 matmul needs 2 half-speed passes, bf16 one;
        # quantization (~0.2% of per-row sums) is far inside the gate.
        rowred = small_pool.tile([P, 1], mybir.dt.bfloat16)
        with nc.allow_low_precision(reason="bf16 rowred: 0.2% of per-row sums vs 2e-2 gate"):
            nc.vector.reduce_sum(rowred[:], e2_x[:], axis=AxisList.X)
        out_t = small_pool.tile([1, 2], F32)
        # PE colsum bank -> out_t[0,1] on ACT (copy with accumulate); the
        # host adds out[0,0] + out[0,1]/C, so the two end chains only meet
        # at the final DMA.
        cs_junk = small_pool.tile([1, 512], F32)
        nc.scalar.activation(
            cs_junk[:], colsum_ps[:], ActFn.Copy, accum_out=out_t[:, 1:2]
        )
        # cross-partition sum via PE: rowred[128,1].T @ ones[128,1] -> [1,1]
        total_ps = psum_pool.tile([1, 1], F32)
        ones_b = small_pool.tile([P, 1], mybir.dt.bfloat16)
        nc.vector.memset(ones_b[:], 1.0)
        nc.tensor.matmul(total_ps[:], rowred[:], ones_b[:], start=True, stop=True)
        nc.vector.tensor_copy(out_t[:, 0:1], total_ps[:])
        nc.sync.dma_start(out[:], out_t[:])


def build_nc() -> bass.Bass:
    nc = bacc.Bacc("TRN2", target_bir_lowering=False, debug=False, num_devices=NCORES)
    pred = nc.dram_tensor("pred", [P, T * C], F8, kind="ExternalInput").ap()
    paux = nc.dram_tensor("paux", [P, NAUX], F32, kind="ExternalInput").ap()
    out = nc.dram_tensor("out", [1, 2], F32, kind="ExternalOutput").ap()
    with tile.TileContext(nc) as tc:
        _loss_program(nc, tc, pred, paux, out)
    nc.compile()
    return nc


def make_in_maps(positive_prob, predictions, target):
    """Shard full inputs into per-core input maps (host-side prep only)."""
    pp_all = np.asarray(positive_prob, dtype=np.float32)
    tg_all = np.asarray(target).astype(np.int64)
    pr_all = np.asarray(predictions, dtype=np.float32)
    M = np.float32(MARGIN)
    in_maps = []
    for i in range(NCORES):
        sl = slice(i * BS, (i + 1) * BS)
        pr = pr_all[sl]                                   # [BS, C] f32
        pr8 = pr.astype(NPF8)
        # [BS, C] -> [P, T*C]: row g*P + p -> partition p, cols [g*C,(g+1)*C)
        pred8 = np.ascontiguousarray(
            pr8.reshape(T, P, C).transpose(1, 0, 2).reshape(P, T * C)
        )
        pp = pp_all[sl].reshape(T, P).T                   # [P, T]
        tg = tg_all[sl]
        pos = (tg != 0).astype(np.float32).reshape(T, P).T
        c2 = 2.0 * pos - 2.0
        y = np.maximum(tg - 1, 0)
        xy = pr[np.arange(BS), y].reshape(T, P).T         # exact f32 [P, T]
        biasA = (M - xy).astype(np.float32)
        # PE path: threshold nb2 pre-rounded to fp8 (so the fp8 junk cast is
        # exact); 8.0 for negative rows (all junk elems become exactly 8.0,
        # the largest power of two under the e3m4 max of 15.5).
        nb2q = (xy - M).astype(NPF8).astype(np.float32)
        nb2_all = np.where(pos > 0, nb2q, np.float32(8.0))
        pe_mask = np.zeros((1, T), dtype=np.float32)
        for g in PE_TILES:
            pe_mask[0, g] = 1.0
        nb2 = nb2_all * pe_mask
        acc0 = np.zeros((P, TX), dtype=np.float32)
        acc0[:, 0:T] = -np.float32(C) * nb2
        # h3 = 0.5*c2*nb2 - pos*M/C  (PE neg-row cancel + margin self-term)
        h3 = np.zeros((P, TX), dtype=np.float32)
        h3[:, 0:T] = 0.5 * c2 * nb2 - pos * (M / np.float32(C))
        pos_ext = np.concatenate([pos, pos[:, SPLIT_TILE : SPLIT_TILE + 1]], axis=1)
        s1cr = (xy - M)[:, SPLIT_TILE : SPLIT_TILE + 1].astype(np.float32)
        s2cr = np.float32(NCR) * (M - xy)[:, SPLIT_TILE : SPLIT_TILE + 1]
        paux = np.ascontiguousarray(
            np.concatenate(
                [pp, pos_ext, c2, biasA, nb2, acc0, h3, s1cr, s2cr], axis=1
            ).astype(np.float32)
        )
        in_maps.append({"pred": pred8, "paux": paux})
    return in_maps


_NC_CACHE = []


def kernel(positive_prob, predictions, target):
    in_maps = make_in_maps(positive_prob, predictions, target)
    if not _NC_CACHE:
        _NC_CACHE.append(build_nc())
    nc = _NC_CACHE[0]
    res = run_bass_kernel_spmd(nc, in_maps, list(range(NCORES)))
    total = np.float32(0.0)
    for r in res.results:
        o = r["out"]
        total += np.float32(o[0, 0]) + np.float32(o[0, 1]) / np.float32(C)
    return np.asarray(total, dtype=np.float32)


# revision 23
# speedup vs baseline: 1.0709x; 1.0084x over previous
"""Trainium2 Bass kernel for CustomCombinedLoss (weighted BCE sum + MultiMarginLoss).

loss = -sum(w * (pos_t*log(p) + (1-pos_t)*log(1-p)))          # w=2 for target==0
     + sum_{i: target_i>0} (1/C) * sum_{j != y_i} max(0, margin - x[i,y_i] + x[i,j])

Sharding: pure data parallel over the batch dim, B=16384 rows -> 8 cores x 2048 rows.
Each core computes a partial scalar loss; host sums the 8 partials.

Design (measured ~30-31us vs the 43-49us fp16 gather-based baseline):
  - predictions stream as fp8 e3m4 (range +-15.5, 4-bit mantissa): halves HBM
    traffic to ~4.2 MB/core, a ~12.5 us DMA floor at ~340 GB/s.  Margin-term
    error from fp8 quantization is ~1e-4 relative (vs the 2e-2 gate).
  - all O(B) per-row parameters (pos, 2*pos-2, margin-xy biases, the PE-path
    max() thresholds and their corrections) are host-precomputed and arrive in
    one small f32 side DMA.  This removes the gpsimd ap_gather xy-extraction
    entirely - whose one-time ~6us Q7 IRAM library load was the baseline's
    critical-path bottleneck (first hinge op could not start until ~16us).
  - hinge work split across three engine paths, balanced to measured fp8
    per-tile costs ([128,2048] tile):
      ACT  (~2.19us): activation(Relu, bias=margin-xy, accum_out) -> row sums
      PE   (~1.2us DVE + ~1.0-1.8us PE): DVE tensor_scalar max -> fp8 junk,
           PE ones-stationary matmuls column-sum the junk into one [1,512]
           PSUM bank accumulated across all PE tiles; per-row corrections are
           host-precomputed constants.  The max() threshold is pre-rounded to
           fp8 on the host so the junk output cast is exact.  The PE ramps to
           its full p-state (216ns/matmul) because the matmul queue stays
           continuously busy.
      CR   (tail of the last tile only): DVE tensor_scalar(max, add)
           cache-reduce, where accum = scalar2_seed + sum_j max(x_j, scalar1)
           (seed semantics, verified in CoreSim).
    The last tile is column-split ACT|CR so both engines finish ~1us after
    the final DMA chunk lands.
  - per-row loss assembled on DVE over a [128, 17] grid (16 tiles + the
    split-CR slot): row_total = (acc/C)*pos + dh, with dh = c2*lq + h3 -
    pos*lp folded early so the post-stream chain is 3 ops + a bf16
    cross-partition matmul.  The PE colsum bank is reduced by one ACT
    copy-accum into out[0,1]; the host adds out[0,0] + out[0,1]/C per core
    so the two end chains only meet at the final [1,2] DMA.
  - DMA: 8 chunks of 2 tiles (512KB).  Every chunk's completion semaphore
    waits on the slowest of the 16 SDMA engines (engine 15 chronically lags
    1.5-2.5us and crawls on its first descriptors), so finer chunking
    multiplies sem-wait bubbles; the first chunk rides the scalar HWDGE ring
    so its descriptor generation runs parallel to paux's on the sync ring.
    A dummy Ln on a memset tile hoists the ~1.3us ACT_TABLE_LOAD off the
    paux critical path.

Layout: row r = g*128 + p of the shard lives at partition p, grid column g.
pred DRAM is [128, 16*2048] fp8: tile g at columns [g*2048, (g+1)*2048).
"""

from contextlib import ExitStack

import ml_dtypes
import numpy as np

import concourse.bacc as bacc
import concourse.bass as bass
import concourse.mybir as mybir
import concourse.tile as tile
from concourse.bass_utils import run_bass_kernel_spmd

WEIGHT = 2.0
MARGIN = 0.5
B, C = 16384, 2048
NCORES = 8
BS = B // NCORES          # rows per core
P = 128                   # partitions
T = BS // P               # row tiles per core (16)
TX = T + 1                # grid cols incl. the split-CR accumulator slot
F32 = mybir.dt.float32
F16 = mybir.dt.float16
F8 = mybir.dt.float8e3
NPF8 = ml_dtypes.float8_e3m4

AluOp = mybir.AluOpType
ActFn = mybir.ActivationFunctionType
AxisList = mybir.AxisListType

# tile -> engine path assignment (tunable)
ACT_TILES = frozenset({0, 3, 6, 9, 12})
PE_TILES = frozenset({1, 2, 4, 5, 7, 8, 10, 11, 13, 14})
CR_TILES = frozenset()
SPLIT_TILE = 15
SPLIT_COL = 1408          # [0:SPLIT_COL] -> ACT, [SPLIT_COL:C] -> CR
NCR = C - SPLIT_COL

# DMA chunk schedule: (first_tile, n_tiles).  Uniform 512KB chunks: each
# chunk's completion semaphore waits on the slowest of the 16 SDMA engines
# (engine 15 chronically lags ~1.5-2.5us), so many small chunks multiply
# sem-wait bubbles while big chunks delay the pipeline head.
CHUNKS = ((0, 2), (2, 2), (4, 2), (6, 2), (8, 2), (10, 2), (12, 2),
          (14, 2))

# paux column offsets (all f32)
O_PPROB = 0               # [T]
O_POS = O_PPROB + T       # [TX]
O_C2 = O_POS + TX         # [T]
O_BIASA = O_C2 + T        # [T]
O_NB2 = O_BIASA + T       # [T]
O_ACC = O_NB2 + T         # [TX]
O_H3 = O_ACC + TX         # [TX]
O_S1CR = O_H3 + TX        # [1]
O_S2CR = O_S1CR + 1       # [1]
NAUX = O_S2CR + 1


def _loss_program(nc: bass.Bass, tc: "tile.TileContext", pred, paux, out):
    ctx = ExitStack()
    with ctx:
        small_pool = ctx.enter_context(tc.tile_pool(name="small", bufs=1))
        pred_pool = ctx.enter_context(tc.tile_pool(name="pred", bufs=len(CHUNKS)))
        psum_pool = ctx.enter_context(tc.tile_pool(name="psum", bufs=1, space="PSUM"))

        # paux rides first on the sync HWDGE ring: it gates every compute op,
        # and on the scalar/ACT ring its completion wait also trapped the
        # second ACT_TABLE_LOAD behind it (first Relu slipped ~2us).
        aux_t = small_pool.tile([P, NAUX], F32)
        nc.sync.dma_start(aux_t[:], paux[:])
        pprob = aux_t[:, O_PPROB : O_PPROB + T]
        pos_x = aux_t[:, O_POS : O_POS + TX]
        c2 = aux_t[:, O_C2 : O_C2 + T]
        acc = aux_t[:, O_ACC : O_ACC + TX]
        h3_x = aux_t[:, O_H3 : O_H3 + TX]

        ones8 = small_pool.tile([P, 1], F8)
        nc.vector.memset(ones8[:], 1.0)
        lp_x = small_pool.tile([P, TX], F32)
        nc.vector.memset(lp_x[:], 0.0)
        dh_x = small_pool.tile([P, TX], F32)
        nc.vector.memset(dh_x[:], 0.0)

        # ---- BCE row terms (off the critical path; Ln first so the ACT
        # table set containing Ln loads once, early).  The dummy Ln on a
        # memset tile has no DMA dependency: walrus places the ~1.3us
        # ACT_TABLE_LOAD right before the first Ln, and without the dummy
        # that position sits behind the paux-DMA semaphore wait.
        warm = small_pool.tile([P, 1], F32)
        nc.vector.memset(warm[:], 1.0)
        warm2 = small_pool.tile([P, 1], F32)
        nc.scalar.activation(warm2[:], warm[:], ActFn.Ln)
        nc.scalar.activation(lp_x[:, 0:T], pprob, ActFn.Ln)
        q_t = small_pool.tile([P, T], F32)
        nc.vector.tensor_scalar(q_t[:], pprob, -1.0, 1.0, AluOp.mult, AluOp.add)
        lq_t = small_pool.tile([P, T], F32)
        nc.scalar.activation(lq_t[:], q_t[:], ActFn.Ln)
        d_t = small_pool.tile([P, T], F32)
        nc.vector.tensor_mul(d_t[:], c2, lq_t[:])
        # dh = c2*lq + h3 - pos*lp, all folded early so the post-stream chain
        # is just t1 = (acc/C)*pos; e2 = t1 + dh; rowsum.  (col T stays h3's 0)
        nc.vector.tensor_add(dh_x[:, 0:T], d_t[:], h3_x[:, 0:T])
        nc.vector.tensor_copy(dh_x[:, T:TX], h3_x[:, T:TX])
        plq_x = small_pool.tile([P, TX], F32)
        nc.vector.tensor_mul(plq_x[:], pos_x, lp_x[:])
        nc.vector.tensor_tensor(dh_x[:], dh_x[:], plq_x[:], AluOp.subtract)

        # scratch outputs (never read)
        jact = small_pool.tile([P, C], F8)
        jcr = small_pool.tile([P, NCR], F16)
        jpes = [
            small_pool.tile([P, C], F8, name=f"jpe{i}") for i in range(3)
        ]

        colsum_ps = psum_pool.tile([1, 512], F32)
        first_pe = min(PE_TILES)
        last_pe = max(PE_TILES)

        # ---- stream chunks
        npe = 0
        for ci, (g0, ntiles) in enumerate(CHUNKS):
            st = pred_pool.tile([P, 2 * C], F8, tag="pred")
            eng = nc.scalar if ci == 0 else nc.sync
            eng.dma_start(
                st[:, 0 : ntiles * C], pred[:, g0 * C : (g0 + ntiles) * C]
            )
            for b in range(ntiles):
                g = g0 + b
                blk = st[:, b * C : (b + 1) * C]
                gg = slice(g, g + 1)
                if g in ACT_TILES:
                    nc.scalar.activation(
                        jact[:], blk, ActFn.Relu,
                        bias=aux_t[:, O_BIASA + g : O_BIASA + g + 1],
                        scale=1.0, accum_out=acc[:, gg],
                    )
                elif g in PE_TILES:
                    jpe = jpes[npe % 3]
                    npe += 1
                    nc.vector.tensor_scalar(
                        jpe[:], blk, aux_t[:, O_NB2 + g : O_NB2 + g + 1],
                        None, AluOp.max,
                    )
                    for k in range(4):
                        csl = slice(k * 512, (k + 1) * 512)
                        nc.tensor.matmul(
                            colsum_ps[:], ones8[:], jpe[:, csl],
                            start=(g == first_pe and k == 0),
                            stop=(g == last_pe and k == 3),
                        )
                elif g in CR_TILES:
                    nc.vector.tensor_scalar(
                        jcr[:, 0:NCR], blk[:, 0:NCR],
                        aux_t[:, O_S1CR : O_S1CR + 1],
                        aux_t[:, O_S2CR : O_S2CR + 1],
                        AluOp.max, AluOp.add, accum_out=acc[:, gg],
                    )
                else:  # SPLIT_TILE
                    nc.scalar.activation(
                        jact[:, 0:SPLIT_COL], blk[:, 0:SPLIT_COL], ActFn.Relu,
                        bias=aux_t[:, O_BIASA + g : O_BIASA + g + 1],
                        scale=1.0, accum_out=acc[:, gg],
                    )
                    nc.vector.tensor_scalar(
                        jcr[:], blk[:, SPLIT_COL:C],
                        aux_t[:, O_S1CR : O_S1CR + 1],
                        aux_t[:, O_S2CR : O_S2CR + 1],
                        AluOp.max, AluOp.add, accum_out=acc[:, T : T + 1],
                    )

        # ---- epilogue
        # row_total = (acc/C)*pos + dh; dh already folds -pos*lp, c2*lq, the
        # -pos*M/C self-term and the PE-path neg-row correction.
        t1_x = small_pool.tile([P, TX], F32)
        nc.vector.scalar_tensor_tensor(
            t1_x[:], acc, 1.0 / C, pos_x, AluOp.mult, AluOp.mult
        )
        e2_x = small_pool.tile([P, TX], F32)
        nc.vector.tensor_add(e2_x[:], t1_x[:], dh_x[:])
        # bf16 rowred: the PE f32 matmul needs 2 half-speed passes, bf16 one;
        # quantization (~0.2% of per-row sums) is far inside the gate.
        rowred = small_pool.tile([P, 1], mybir.dt.bfloat16)
        with nc.allow_low_precision(reason="bf16 rowred: 0.2% of per-row sums vs 2e-2 gate"):
            nc.vector.reduce_sum(rowred[:], e2_x[:], axis=AxisList.X)
        out_t = small_pool.tile([1, 2], F32)
        # PE colsum bank -> out_t[0,1] on ACT (copy with accumulate); the
        # host adds out[0,0] + out[0,1]/C, so the two end chains only meet
        # at the final DMA.
        cs_junk = small_pool.tile([1, 512], F32)
        nc.scalar.activation(
            cs_junk[:], colsum_ps[:], ActFn.Copy, accum_out=out_t[:, 1:2]
        )
        # cross-partition sum via PE: rowred[128,1].T @ ones[128,1] -> [1,1]
        total_ps = psum_pool.tile([1, 1], F32)
        ones_b = small_pool.tile([P, 1], mybir.dt.bfloat16)
        nc.vector.memset(ones_b[:], 1.0)
        nc.tensor.matmul(total_ps[:], rowred[:], ones_b[:], start=True, stop=True)
        nc.vector.tensor_copy(out_t[:, 0:1], total_ps[:])
        nc.sync.dma_start(out[:], out_t[:])


def build_nc() -> bass.Bass:
    nc = bacc.Bacc("TRN2", target_bir_lowering=False, debug=False, num_devices=NCORES)
    pred = nc.dram_tensor("pred", [P, T * C], F8, kind="ExternalInput").ap()
    paux = nc.dram_tensor("paux", [P, NAUX], F32, kind="ExternalInput").ap()
    out = nc.dram_tensor("out", [1, 2], F32, kind="ExternalOutput").ap()
    with tile.TileContext(nc) as tc:
        _loss_program(nc, tc, pred, paux, out)
    nc.compile()
    return nc


def make_in_maps(positive_prob, predictions, target):
    """Shard full inputs into per-core input maps (host-side prep only)."""
    pp_all = np.asarray(positive_prob, dtype=np.float32)
    tg_all = np.asarray(target).astype(np.int64)
    pr_all = np.asarray(predictions, dtype=np.float32)
    M = np.float32(MARGIN)
    in_maps = []
    for i in range(NCORES):
        sl = slice(i * BS, (i + 1) * BS)
        pr = pr_all[sl]                                   # [BS, C] f32
        pr8 = pr.astype(NPF8)
        # [BS, C] -> [P, T*C]: row g*P + p -> partition p, cols [g*C,(g+1)*C)
        pred8 = np.ascontiguousarray(
            pr8.reshape(T, P, C).transpose(1, 0, 2).reshape(P, T * C)
        )
        pp = pp_all[sl].reshape(T, P).T                   # [P, T]
        tg = tg_all[sl]
        pos = (tg != 0).astype(np.float32).reshape(T, P).T
        c2 = 2.0 * pos - 2.0
        y = np.maximum(tg - 1, 0)
        xy = pr[np.arange(BS), y].reshape(T, P).T         # exact f32 [P, T]
        biasA = (M - xy).astype(np.float32)
        # PE path: threshold nb2 pre-rounded to fp8 (so the fp8 junk cast is
        # exact); 8.0 for negative rows (all junk elems become exactly 8.0,
        # the largest power of two under the e3m4 max of 15.5).
        nb2q = (xy - M).astype(NPF8).astype(np.float32)
        nb2_all = np.where(pos > 0, nb2q, np.float32(8.0))
        pe_mask = np.zeros((1, T), dtype=np.float32)
        for g in PE_TILES:
            pe_mask[0, g] = 1.0
        nb2 = nb2_all * pe_mask
        acc0 = np.zeros((P, TX), dtype=np.float32)
        acc0[:, 0:T] = -np.float32(C) * nb2
        # h3 = 0.5*c2*nb2 - pos*M/C  (PE neg-row cancel + margin self-term)
        h3 = np.zeros((P, TX), dtype=np.float32)
        h3[:, 0:T] = 0.5 * c2 * nb2 - pos * (M / np.float32(C))
        pos_ext = np.concatenate([pos, pos[:, SPLIT_TILE : SPLIT_TILE + 1]], axis=1)
        s1cr = (xy - M)[:, SPLIT_TILE : SPLIT_TILE + 1].astype(np.float32)
        s2cr = np.float32(NCR) * (M - xy)[:, SPLIT_TILE : SPLIT_TILE + 1]
        paux = np.ascontiguousarray(
            np.concatenate(
                [pp, pos_ext, c2, biasA, nb2, acc0, h3, s1cr, s2cr], axis=1
            ).astype(np.float32)
        )
        in_maps.append({"pred": pred8, "paux": paux})
    return in_maps


_NC_CACHE = []


def kernel(positive_prob, predictions, target):
    in_maps = make_in_maps(positive_prob, predictions, target)
    if not _NC_CACHE:
        _NC_CACHE.append(build_nc())
    nc = _NC_CACHE[0]
    res = run_bass_kernel_spmd(nc, in_maps, list(range(NCORES)))
    total = np.float32(0.0)
    for r in res.results:
        o = r["out"]
        total += np.float32(o[0, 0]) + np.float32(o[0, 1]) / np.float32(C)
    return np.asarray(total, dtype=np.float32)


# revision 26
# speedup vs baseline: 1.0748x; 1.0036x over previous
"""Trainium2 Bass kernel for CustomCombinedLoss (weighted BCE sum + MultiMarginLoss).

loss = -sum(w * (pos_t*log(p) + (1-pos_t)*log(1-p)))          # w=2 for target==0
     + sum_{i: target_i>0} (1/C) * sum_{j != y_i} max(0, margin - x[i,y_i] + x[i,j])

Sharding: pure data parallel over the batch dim, B=16384 rows -> 8 cores x 2048 rows.
Each core computes a partial scalar loss; host sums the 8 partials.

Design (measured ~30-31us vs the 43-49us fp16 gather-based baseline):
  - predictions stream as fp8 e3m4 (range +-15.5, 4-bit mantissa): halves HBM
    traffic to ~4.2 MB/core, a ~12.5 us DMA floor at ~340 GB/s.  Margin-term
    error from fp8 quantization is ~1e-4 relative (vs the 2e-2 gate).
  - all O(B) per-row parameters (pos, 2*pos-2, margin-xy biases, the PE-path
    max() thresholds and their corrections) are host-precomputed and arrive in
    one small f32 side DMA.  This removes the gpsimd ap_gather xy-extraction
    entirely - whose one-time ~6us Q7 IRAM library load was the baseline's
    critical-path bottleneck (first hinge op could not start until ~16us).
  - hinge work split across three engine paths, balanced to measured fp8
    per-tile costs ([128,2048] tile):
      ACT  (~2.19us): activation(Relu, bias=margin-xy, accum_out) -> row sums
      PE   (~1.2us DVE + ~1.0-1.8us PE): DVE tensor_scalar max -> fp8 junk,
           PE ones-stationary matmuls column-sum the junk into one [1,512]
           PSUM bank accumulated across all PE tiles; per-row corrections are
           host-precomputed constants.  The max() threshold is pre-rounded to
           fp8 on the host so the junk output cast is exact.  The PE ramps to
           its full p-state (216ns/matmul) because the matmul queue stays
           continuously busy.
      CR   (tail of the last tile only): DVE tensor_scalar(max, add)
           cache-reduce, where accum = scalar2_seed + sum_j max(x_j, scalar1)
           (seed semantics, verified in CoreSim).
    The last tile is column-split ACT|CR so both engines finish ~1us after
    the final DMA chunk lands.
  - per-row loss assembled on DVE over a [128, 17] grid (16 tiles + the
    split-CR slot): row_total = (acc/C)*pos + dh, with dh = c2*lq + h3 -
    pos*lp folded early so the post-stream chain is 3 ops + a bf16
    cross-partition matmul.  The PE colsum bank is reduced by one ACT
    copy-accum into out[0,1]; the host adds out[0,0] + out[0,1]/C per core
    so the two end chains only meet at the final [1,2] DMA.
  - DMA: 8 chunks of 2 tiles (512KB).  Every chunk's completion semaphore
    waits on the slowest of the 16 SDMA engines (engine 15 chronically lags
    1.5-2.5us and crawls on its first descriptors), so finer chunking
    multiplies sem-wait bubbles; the first chunk rides the scalar HWDGE ring
    so its descriptor generation runs parallel to paux's on the sync ring.
    A dummy Ln on a memset tile hoists the ~1.3us ACT_TABLE_LOAD off the
    paux critical path.

Layout: row r = g*128 + p of the shard lives at partition p, grid column g.
pred DRAM is [128, 16*2048] fp8: tile g at columns [g*2048, (g+1)*2048).
"""

from contextlib import ExitStack

import ml_dtypes
import numpy as np

import concourse.bacc as bacc
import concourse.bass as bass
import concourse.mybir as mybir
import concourse.tile as tile
from concourse.bass_utils import run_bass_kernel_spmd

WEIGHT = 2.0
MARGIN = 0.5
B, C = 16384, 2048
NCORES = 8
BS = B // NCORES          # rows per core
P = 128                   # partitions
T = BS // P               # row tiles per core (16)
TX = T + 1                # grid cols incl. the split-CR accumulator slot
F32 = mybir.dt.float32
F16 = mybir.dt.float16
F8 = mybir.dt.float8e3
NPF8 = ml_dtypes.float8_e3m4

AluOp = mybir.AluOpType
ActFn = mybir.ActivationFunctionType
AxisList = mybir.AxisListType

# tile -> engine path assignment (tunable)
ACT_TILES = frozenset({0, 3, 6, 9, 12})
PE_TILES = frozenset({1, 2, 4, 5, 7, 8, 10, 11, 13, 14})
CR_TILES = frozenset()
SPLIT_TILE = 15
SPLIT_COL = 1408          # [0:SPLIT_COL] -> ACT, [SPLIT_COL:C] -> CR
NCR = C - SPLIT_COL

# DMA chunk schedule: (first_tile, n_tiles).  Uniform 512KB chunks: each
# chunk's completion semaphore waits on the slowest of the 16 SDMA engines
# (engine 15 chronically lags ~1.5-2.5us), so many small chunks multiply
# sem-wait bubbles while big chunks delay the pipeline head.
CHUNKS = ((0, 2), (2, 2), (4, 2), (6, 2), (8, 2), (10, 2), (12, 2),
          (14, 2))

# paux column offsets (all f32)
O_PPROB = 0               # [T]
O_POS = O_PPROB + T       # [TX]
O_C2 = O_POS + TX         # [T]
O_BIASA = O_C2 + T        # [T]
O_NB2 = O_BIASA + T       # [T]
O_ACC = O_NB2 + T         # [TX]
O_H3 = O_ACC + TX         # [TX]
O_S1CR = O_H3 + TX        # [1]
O_S2CR = O_S1CR + 1       # [1]
NAUX = O_S2CR + 1


def _loss_program(nc: bass.Bass, tc: "tile.TileContext", pred, paux, out):
    ctx = ExitStack()
    with ctx:
        small_pool = ctx.enter_context(tc.tile_pool(name="small", bufs=1))
        pred_pool = ctx.enter_context(tc.tile_pool(name="pred", bufs=len(CHUNKS)))
        psum_pool = ctx.enter_context(tc.tile_pool(name="psum", bufs=1, space="PSUM"))

        # paux rides first on the sync HWDGE ring: it gates every compute op,
        # and on the scalar/ACT ring its completion wait also trapped the
        # second ACT_TABLE_LOAD behind it (first Relu slipped ~2us).
        aux_t = small_pool.tile([P, NAUX], F32)
        nc.sync.dma_start(aux_t[:], paux[:])
        pprob = aux_t[:, O_PPROB : O_PPROB + T]
        pos_x = aux_t[:, O_POS : O_POS + TX]
        c2 = aux_t[:, O_C2 : O_C2 + T]
        acc = aux_t[:, O_ACC : O_ACC + TX]
        h3_x = aux_t[:, O_H3 : O_H3 + TX]

        ones8 = small_pool.tile([P, 1], F8)
        nc.vector.memset(ones8[:], 1.0)
        lp_x = small_pool.tile([P, TX], F32)
        nc.vector.memset(lp_x[:], 0.0)
        dh_x = small_pool.tile([P, TX], F32)
        nc.vector.memset(dh_x[:], 0.0)

        # ---- BCE row terms (off the critical path; Ln first so the ACT
        # table set containing Ln loads once, early).  The dummy Ln on a
        # memset tile has no DMA dependency: walrus places the ~1.3us
        # ACT_TABLE_LOAD right before the first Ln, and without the dummy
        # that position sits behind the paux-DMA semaphore wait.
        warm = small_pool.tile([P, 1], F32)
        nc.vector.memset(warm[:], 1.0)
        warm2 = small_pool.tile([P, 1], F32)
        nc.scalar.activation(warm2[:], warm[:], ActFn.Ln)
        nc.scalar.activation(lp_x[:, 0:T], pprob, ActFn.Ln)
        q_t = small_pool.tile([P, T], F32)
        nc.vector.tensor_scalar(q_t[:], pprob, -1.0, 1.0, AluOp.mult, AluOp.add)
        lq_t = small_pool.tile([P, T], F32)
        nc.scalar.activation(lq_t[:], q_t[:], ActFn.Ln)
        d_t = small_pool.tile([P, T], F32)
        nc.vector.tensor_mul(d_t[:], c2, lq_t[:])
        # dh = c2*lq + h3 - pos*lp, all folded early so the post-stream chain
        # is just t1 = (acc/C)*pos; e2 = t1 + dh; rowsum.  (col T stays h3's 0)
        nc.vector.tensor_add(dh_x[:, 0:T], d_t[:], h3_x[:, 0:T])
        nc.vector.tensor_copy(dh_x[:, T:TX], h3_x[:, T:TX])
        plq_x = small_pool.tile([P, TX], F32)
        nc.vector.tensor_mul(plq_x[:], pos_x, lp_x[:])
        nc.vector.tensor_tensor(dh_x[:], dh_x[:], plq_x[:], AluOp.subtract)

        # scratch outputs (never read)
        jact = small_pool.tile([P, C], F8)
        jcr = small_pool.tile([P, NCR], F16)
        jpes = [
            small_pool.tile([P, C], F8, name=f"jpe{i}") for i in range(3)
        ]

        colsum_ps = psum_pool.tile([1, 512], F32)
        first_pe = min(PE_TILES)
        last_pe = max(PE_TILES)

        # ---- stream chunks
        npe = 0
        for ci, (g0, ntiles) in enumerate(CHUNKS):
            st = pred_pool.tile([P, 2 * C], F8, tag="pred")
            eng = nc.scalar if ci == 0 else nc.sync
            eng.dma_start(
                st[:, 0 : ntiles * C], pred[:, g0 * C : (g0 + ntiles) * C]
            )
            for b in range(ntiles):
                g = g0 + b
                blk = st[:, b * C : (b + 1) * C]
                gg = slice(g, g + 1)
                if g in ACT_TILES:
                    nc.scalar.activation(
                        jact[:], blk, ActFn.Relu,
                        bias=aux_t[:, O_BIASA + g : O_BIASA + g + 1],
                        scale=1.0, accum_out=acc[:, gg],
                    )
                elif g in PE_TILES:
                    jpe = jpes[npe % 3]
                    npe += 1
                    nc.vector.tensor_scalar(
                        jpe[:], blk, aux_t[:, O_NB2 + g : O_NB2 + g + 1],
                        None, AluOp.max,
                    )
                    for k in range(4):
                        csl = slice(k * 512, (k + 1) * 512)
                        nc.tensor.matmul(
                            colsum_ps[:], ones8[:], jpe[:, csl],
                            start=(g == first_pe and k == 0),
                            stop=(g == last_pe and k == 3),
                        )
                elif g in CR_TILES:
                    nc.vector.tensor_scalar(
                        jcr[:, 0:NCR], blk[:, 0:NCR],
                        aux_t[:, O_S1CR : O_S1CR + 1],
                        aux_t[:, O_S2CR : O_S2CR + 1],
                        AluOp.max, AluOp.add, accum_out=acc[:, gg],
                    )
                else:  # SPLIT_TILE
                    nc.scalar.activation(
                        jact[:, 0:SPLIT_COL], blk[:, 0:SPLIT_COL], ActFn.Relu,
                        bias=aux_t[:, O_BIASA + g : O_BIASA + g + 1],
                        scale=1.0, accum_out=acc[:, gg],
                    )
                    nc.vector.tensor_scalar(
                        jcr[:], blk[:, SPLIT_COL:C],
                        aux_t[:, O_S1CR : O_S1CR + 1],
                        aux_t[:, O_S2CR : O_S2CR + 1],
                        AluOp.max, AluOp.add, accum_out=acc[:, T : T + 1],
                    )

        # ---- epilogue
        # row_total = (acc/C)*pos + dh; dh already folds -pos*lp, c2*lq, the
        # -pos*M/C self-term and the PE-path neg-row correction.
        t1_x = small_pool.tile([P, TX], F32)
        nc.vector.scalar_tensor_tensor(
            t1_x[:], acc, 1.0 / C, pos_x, AluOp.mult, AluOp.mult
        )
        e2_x = small_pool.tile([P, TX], F32)
        nc.vector.tensor_add(e2_x[:], t1_x[:], dh_x[:])
        # bf16 rowred: the PE f32 matmul needs 2 half-speed passes, bf16 one;
        # quantization (~0.2% of per-row sums) is far inside the gate.
        rowred = small_pool.tile([P, 1], mybir.dt.bfloat16)
        with nc.allow_low_precision(reason="bf16 rowred: 0.2% of per-row sums vs 2e-2 gate"):
            nc.vector.reduce_sum(rowred[:], e2_x[:], axis=AxisList.X)
        out_t = small_pool.tile([1, 2], F32)
        # PE colsum bank -> out_t[0,1] on ACT (copy with accumulate); the
        # host adds out[0,0] + out[0,1]/C, so the two end chains only meet
        # at the final DMA.
        cs_junk = small_pool.tile([1, 512], F32)
        nc.scalar.activation(
            cs_junk[:], colsum_ps[:], ActFn.Copy, accum_out=out_t[:, 1:2]
        )
        # cross-partition sum via PE: rowred[128,1].T @ ones[128,1] -> [1,1]
        total_ps = psum_pool.tile([1, 1], F32)
        ones_b = small_pool.tile([P, 1], mybir.dt.bfloat16)
        nc.vector.memset(ones_b[:], 1.0)
        nc.tensor.matmul(total_ps[:], rowred[:], ones_b[:], start=True, stop=True)
        nc.vector.tensor_copy(out_t[:, 0:1], total_ps[:])
        nc.sync.dma_start(out[:], out_t[:])


def build_nc() -> bass.Bass:
    nc = bacc.Bacc("TRN2", target_bir_lowering=False, debug=False, num_devices=NCORES)
    pred = nc.dram_tensor("pred", [P, T * C], F8, kind="ExternalInput").ap()
    paux = nc.dram_tensor("paux", [P, NAUX], F32, kind="ExternalInput").ap()
    out = nc.dram_tensor("out", [1, 2], F32, kind="ExternalOutput").ap()
    with tile.TileContext(nc) as tc:
        _loss_program(nc, tc, pred, paux, out)
    nc.compile()
    return nc


def make_in_maps(positive_prob, predictions, target):
    """Shard full inputs into per-core input maps (host-side prep only)."""
    pp_all = np.asarray(positive_prob, dtype=np.float32)
    tg_all = np.asarray(target).astype(np.int64)
    pr_all = np.asarray(predictions, dtype=np.float32)
    M = np.float32(MARGIN)
    in_maps = []
    for i in range(NCORES):
        sl = slice(i * BS, (i + 1) * BS)
        pr = pr_all[sl]                                   # [BS, C] f32
        pr8 = pr.astype(NPF8)
        # [BS, C] -> [P, T*C]: row g*P + p -> partition p, cols [g*C,(g+1)*C)
        pred8 = np.ascontiguousarray(
            pr8.reshape(T, P, C).transpose(1, 0, 2).reshape(P, T * C)
        )
        pp = pp_all[sl].reshape(T, P).T                   # [P, T]
        tg = tg_all[sl]
        pos = (tg != 0).astype(np.float32).reshape(T, P).T
        c2 = 2.0 * pos - 2.0
        y = np.maximum(tg - 1, 0)
        xy = pr[np.arange(BS), y].reshape(T, P).T         # exact f32 [P, T]
        biasA = (M - xy).astype(np.float32)
        # PE path: threshold nb2 pre-rounded to fp8 (so the fp8 junk cast is
        # exact); 8.0 for negative rows (all junk elems become exactly 8.0,
        # the largest power of two under the e3m4 max of 15.5).
        nb2q = (xy - M).astype(NPF8).astype(np.float32)
        nb2_all = np.where(pos > 0, nb2q, np.float32(8.0))
        pe_mask = np.zeros((1, T), dtype=np.float32)
        for g in PE_TILES:
            pe_mask[0, g] = 1.0
        nb2 = nb2_all * pe_mask
        acc0 = np.zeros((P, TX), dtype=np.float32)
        acc0[:, 0:T] = -np.float32(C) * nb2
        # h3 = 0.5*c2*nb2 - pos*M/C  (PE neg-row cancel + margin self-term)
        h3 = np.zeros((P, TX), dtype=np.float32)
        h3[:, 0:T] = 0.5 * c2 * nb2 - pos * (M / np.float32(C))
        pos_ext = np.concatenate([pos, pos[:, SPLIT_TILE : SPLIT_TILE + 1]], axis=1)
        s1cr = (xy - M)[:, SPLIT_TILE : SPLIT_TILE + 1].astype(np.float32)
        s2cr = np.float32(NCR) * (M - xy)[:, SPLIT_TILE : SPLIT_TILE + 1]
        paux = np.ascontiguousarray(
            np.concatenate(
                [pp, pos_ext, c2, biasA, nb2, acc0, h3, s1cr, s2cr], axis=1
            ).astype(np.float32)
        )
        in_maps.append({"pred": pred8, "paux": paux})
    return in_maps


_NC_CACHE = []


def kernel(positive_prob, predictions, target):
    in_maps = make_in_maps(positive_prob, predictions, target)
    if not _NC_CACHE:
        _NC_CACHE.append(build_nc())
    nc = _NC_CACHE[0]
    res = run_bass_kernel_spmd(nc, in_maps, list(range(NCORES)))
    total = np.float32(0.0)
    for r in res.results:
        o = r["out"]
        total += np.float32(o[0, 0]) + np.float32(o[0, 1]) / np.float32(C)
    return np.asarray(total, dtype=np.float32)
